# revision 5
# baseline (speedup 1.0000x reference)
"""AttnBlock on 8 Trainium2 NeuronCores via Bass/Tile.

Reference computation (shapes hardcoded): x (4, 256, 64, 64) f32,
GroupNorm(32 groups) -> q/k/v 1x1 conv -> HWxHW attention (with the
reference's raw-view reshape (C,N)->(N,C) for q and v) -> proj -> x + p.

Sharding: 8 cores = 4 batch elements x 2 query-halves, mesh (pair=4,
half=2). Core (b, j) handles batch b and attention rows n in
[j*2048, (j+1)*2048). The raw view means q_att rows [j*2048,(j+1)*2048)
depend only on wq rows [j*128,(j+1)*128), so each core computes: full
k/v, its half of qT, its half of the attention, and p columns
[j*2048,(j+1)*2048). No collectives.

GroupNorm is FOLDED into the q/k/v conv weights: h = scale_c*x + bias_c
per channel, so W@h = (W*scale)@x + W_s@(bias_c/scale). The per-channel
scale is multiplied into the weight converts (same op count as a plain
convert) and the bias terms become tiny matmuls, so h is never
materialized and the convs consume x directly. The q-side bias is per
out-column of the qT tiles and is added during the PSUM->SBUF copies via
scalar_tensor_tensor with a partition-broadcast bq row.

Key layout identity (N=4096=HW, C=256): q_att[n,c] = q[n//16, 256*(n%16)+c],
so  qT[c, 16a+r] = (x[:, 256r:256r+256].T @ wq_half_s.T)[c, a]
which lets us build q_att.T (c on partitions) directly with matmuls.

Attention is computed transposed: ST[j,i] = sum_c k[c,j]*qT[c,i], then
E = exp(ST/16 - 4) (scores are ~N(0,1): no max subtraction needed, and
the -4 centers E in fp8 range and cancels in the normalization),
h_attT[c,i] = sum_j v_att[j,c]*E[j,i] accumulated in PSUM over j-block
pairs. k/qT/E/v_att are fp8(e4m3) with K=256 packed [128,2,.] for
DoubleRow matmuls. The softmax denominator accumulates via M=1
ones-matmuls and is shipped to the host as a second (tiny) output; the
host performs p/l + bp during the unshard, so the device ships the
unnormalized wp @ h_attT in bf16 (bf16 is floating so the larger
magnitude costs no relative precision). In the attention phase ACT does
nothing but the exps (the chain of 64 exps is the co-bottleneck with the
PE's matmul stream), all PSUM->SBUF traffic runs on DVE, and the
accumulation matmuls run one pair behind the score matmuls so the PE
never waits on exp.

Host I/O is minimized: x ships bf16, the big weights ship bf16
(wk|wv|wp transposed), one-hots f32, per-half q weights per half.
Outputs are the bf16 p-halves plus the f32 denominator rows.
"""

import numpy as np

B, C, H, W = 4, 256, 64, 64
N = H * W            # 4096 pixels
HALF = N // 2        # 2048 attention rows per core
GROUPS = 32
GSIZE = C // GROUPS  # 8 channels per group
EPS = 1e-5
NCORES = 8
P = 128              # partitions
NB = N // P          # 32 j-blocks of 128
SC = 4               # i super-chunks per core
CHUNK = 512          # i columns per chunk (one PSUM bank)

# wbig column layout (bf16, C rows): wk.T | wv.T | wp.T
WK0, WV0, WP0 = 0, C, 2 * C
WBCOLS = 3 * C
# wsml column layout (f32, C rows)
BK0, BV0, BP0, GW0, GB0 = 0, 1, 2, 3, 4
IND0 = 5
WSCOLS = IND0 + GROUPS

_cache = {}


def _build_nc():
    import concourse.tile as tile
    from concourse import bacc, mybir

    f32 = mybir.dt.float32
    bf16 = mybir.dt.bfloat16
    f8 = mybir.dt.float8e4
    AF = mybir.ActivationFunctionType
    OP = mybir.AluOpType

    nc = bacc.Bacc("TRN2", target_bir_lowering=False, debug=False,
                   num_devices=NCORES)

    x_ap = nc.dram_tensor("x", [C, N], bf16, kind="ExternalInput").ap()
    qpack_ap = nc.dram_tensor("qpack", [C + 1, P], bf16,
                              kind="ExternalInput").ap()
    wbig_ap = nc.dram_tensor("wbig", [C, WBCOLS], bf16,
                             kind="ExternalInput").ap()
    wsml_ap = nc.dram_tensor("wsml", [C, WSCOLS], f32,
                             kind="ExternalInput").ap()
    windt_ap = nc.dram_tensor("windt", [GROUPS, C], f32,
                              kind="ExternalInput").ap()
    out_ap = nc.dram_tensor("out", [C, HALF], bf16, kind="ExternalOutput").ap()
    lout_ap = nc.dram_tensor("lout", [1, HALF], f32,
                             kind="ExternalOutput").ap()

    with tile.TileContext(nc) as tc:
        with (
            tc.tile_pool(name="persist", bufs=1) as persist,
            tc.tile_pool(name="small", bufs=4) as small,
            tc.tile_pool(name="epool", bufs=4) as epool,
            tc.tile_pool(name="htpool", bufs=4) as htpool,
            tc.tile_pool(name="opool", bufs=4) as opool,
        ):
            # constants first so nothing queues behind the big DMAs
            warm_w = persist.tile([P, P], bf16, tag="warmw", name="warmw")
            nc.vector.memset(warm_w[:], 1.0)
            ones_pair_f8 = persist.tile([P, 2, 16], f8, tag="ones_pair",
                                        name="ones_pair")
            nc.vector.memset(ones_pair_f8[:], 1.0)
            eps_sb = persist.tile([GROUPS, 1], f32, tag="eps", name="eps")
            nc.vector.memset(eps_sb[:], EPS)
            neg4_sb = persist.tile([P, 1], f32, tag="neg4", name="neg4")
            nc.vector.memset(neg4_sb[:], -4.0)

            # ---------- x + weights across the three DMA queues ----------
            # Per-queue DMA bandwidth is ~73GB/s, so balance bytes: sync
            # and scalar take 3 x-chunks each plus the small packs; gpsimd
            # takes the last 2 x-chunks then the big weights (needed only
            # after the GN stats, which wait on all of x anyway).
            x_sb = [persist.tile([P, N], bf16, tag=f"x{cb}", name=f"x{cb}")
                    for cb in range(2)]
            x_chunks = [(0, 0), (1, 0), (0, 1), (1, 1),
                        (0, 2), (1, 2), (0, 3), (1, 3)]
            x_q = (nc.sync, nc.scalar, nc.sync, nc.scalar,
                   nc.sync, nc.scalar, nc.gpsimd, nc.gpsimd)
            for (cb, q), eng in zip(x_chunks, x_q):
                csl = slice(q * 1024, (q + 1) * 1024)
                eng.dma_start(x_sb[cb][:, csl],
                              x_ap[cb * P:(cb + 1) * P, csl])

            def rows(cb):
                return slice(cb * P, (cb + 1) * P)

            wbig_sb, qp_sb = [], []
            for cb in range(2):
                t = persist.tile([P, WBCOLS], bf16, tag=f"wb{cb}",
                                 name=f"wb{cb}")
                nc.gpsimd.dma_start(t[:], wbig_ap[rows(cb), :])
                wbig_sb.append(t)
                t = persist.tile([P, P], bf16, tag=f"qp{cb}", name=f"qp{cb}")
                nc.sync.dma_start(t[:], qpack_ap[rows(cb), :])
                qp_sb.append(t)
            bq_row_bf = persist.tile([1, P], bf16, tag="bqrow", name="bqrow")
            nc.sync.dma_start(bq_row_bf[:], qpack_ap[C:C + 1, :])
            wsml_sb = []
            for cb in range(2):
                t = persist.tile([P, WSCOLS], f32, tag=f"ws{cb}",
                                 name=f"ws{cb}")
                nc.scalar.dma_start(t[:], wsml_ap[rows(cb), :])
                wsml_sb.append(t)
            windt_sb = persist.tile([GROUPS, C], f32, tag="windt",
                                    name="windt")
            nc.scalar.dma_start(windt_sb[:], windt_ap[:, :])

            def wsml(cb, c0, c1=None):
                c1 = c0 + 1 if c1 is None else c1
                return wsml_sb[cb][:, c0:c1]

            wpt_bf = [wbig_sb[cb][:, WP0:WP0 + C] for cb in range(2)]

            DR = mybir.MatmulPerfMode.DoubleRow

            # persistent fp8 tensors
            k_f8 = persist.tile([P, 2, N], f8, tag="kf8", name="kf8")
            v_f8 = [persist.tile([P, N], f8, tag=f"vf8{cb}", name=f"vf8{cb}")
                    for cb in range(2)]
            v_att = persist.tile([P, NB // 2, 2, C], f8, tag="vatt",
                                 name="vatt")
            qT = persist.tile([P, 2, HALF], f8, tag="qT", name="qT")
            qTv = qT.rearrange("p b (a r) -> p b a r", r=16)

            # ---------- pre-attention PSUM: two [P,4,512] quads ----------
            with tc.tile_pool(name="ps4", bufs=2, space="PSUM") as ps4:
                def quad():
                    return ps4.tile([P, 4, CHUNK], f32, tag="q4", name="q4")

                # dummy matmuls consuming each x chunk as it arrives keep
                # the HAM clock-gate at full rate into the convs
                def junk(cb, ch):
                    jq = quad()
                    nc.tensor.matmul(
                        jq[:, 0, :], warm_w[:],
                        x_sb[cb][:, ch * CHUNK:(ch + 1) * CHUNK],
                        start=True, stop=True)

                for rep in range(2):
                    for q in range(4):
                        for cb in range(2):
                            junk(cb, 2 * q + rep)

                # ---------- GroupNorm stats ----------
                m1m2 = []
                for cb in range(2):
                    xv = x_sb[cb].rearrange("p (s q) -> p s q", q=512)
                    stats = small.tile([P, 8, 6], f32, tag="bnstats",
                                       name="bnstats")
                    for s in range(8):
                        nc.vector.bn_stats(stats[:, s, :], xv[:, s, :])
                    mv = small.tile([P, 2], f32, tag="bnmv", name="bnmv")
                    nc.vector.bn_aggr(mv[:], stats[:])
                    mm12 = small.tile([P, 2], f32, tag="m1m2", name="m1m2")
                    nc.vector.tensor_copy(mm12[:, 0:1], mv[:, 0:1])
                    sq = small.tile([P, 1], f32, tag="gnsq", name="gnsq")
                    nc.vector.tensor_mul(sq[:], mv[:, 0:1], mv[:, 0:1])
                    nc.vector.tensor_add(mm12[:, 1:2], mv[:, 1:2], sq[:])
                    m1m2.append(mm12)

                gq = quad()
                # group sums: [32, 2] = sum over channels in group
                nc.tensor.matmul(gq[0:GROUPS, 0, 0:2],
                                 wsml(0, IND0, IND0 + GROUPS),
                                 m1m2[0][:], start=True, stop=False)
                nc.tensor.matmul(gq[0:GROUPS, 0, 0:2],
                                 wsml(1, IND0, IND0 + GROUPS),
                                 m1m2[1][:], start=False, stop=True)

                gstats = small.tile([GROUPS, 2], f32, tag="gstats",
                                    name="gstats")
                nc.vector.tensor_scalar_mul(gstats[:, 0:1],
                                            gq[0:GROUPS, 0, 0:1],
                                            1.0 / GSIZE)
                ex2 = small.tile([GROUPS, 1], f32, tag="gex2", name="gex2")
                nc.vector.tensor_scalar_mul(ex2[:], gq[0:GROUPS, 0, 1:2],
                                            1.0 / GSIZE)
                musq = small.tile([GROUPS, 1], f32, tag="gmusq", name="gmusq")
                nc.vector.tensor_mul(musq[:], gstats[:, 0:1], gstats[:, 0:1])
                gvar = small.tile([GROUPS, 1], f32, tag="gvar", name="gvar")
                nc.vector.tensor_sub(gvar[:], ex2[:], musq[:])
                gsd = small.tile([GROUPS, 1], f32, tag="gsd", name="gsd")
                nc.scalar.activation(gsd[:], gvar[:], AF.Sqrt, bias=eps_sb[:])
                nc.vector.reciprocal(gstats[:, 1:2], gsd[:])

                junk(0, 1)

                # scatter group stats to channels; per-channel fold params
                scale_c, bias2_bf = [], []
                sq2 = quad()
                for cb in range(2):
                    nc.tensor.matmul(sq2[:, cb, 0:2],
                                     windt_sb[:, cb * P:(cb + 1) * P],
                                     gstats[:], start=True, stop=True)
                    sc_ = small.tile([P, 1], f32, tag="scalec", name="scalec")
                    nc.vector.tensor_mul(sc_[:], sq2[:, cb, 1:2],
                                         wsml(cb, GW0))
                    mus = small.tile([P, 1], f32, tag="mus", name="mus")
                    nc.vector.tensor_mul(mus[:], sq2[:, cb, 0:1], sc_[:])
                    bias_c = small.tile([P, 1], f32, tag="biasc", name="biasc")
                    nc.vector.tensor_sub(bias_c[:], wsml(cb, GB0), mus[:])
                    # bias2 = bias_c / scale_c  (so W_s @ bias2 = W @ bias_c)
                    rsc = small.tile([P, 1], f32, tag="rsc", name="rsc")
                    nc.vector.reciprocal(rsc[:], sc_[:])
                    b2 = small.tile([P, 1], f32, tag="b2", name="b2")
                    nc.vector.tensor_mul(b2[:], bias_c[:], rsc[:])
                    b2b = small.tile([P, 1], bf16, tag="b2b", name="b2b")
                    nc.vector.tensor_copy(b2b[:], b2[:])
                    scale_c.append(sc_)
                    bias2_bf.append(b2b)

                junk(1, 1)

                # folded bf16 weights: W_s = W.T * scale_c (per partition)
                def fold(src_ap, cols, tag, cb, eng):
                    t = persist.tile([P, cols], bf16, tag=tag, name=tag)
                    if eng == "dve":
                        nc.vector.tensor_scalar_mul(t[:], src_ap,
                                                    scale_c[cb][:])
                    else:
                        nc.scalar.activation(t[:], src_ap, AF.Identity,
                                             scale=scale_c[cb][:])
                    return t

                wqt_s = [fold(qp_sb[cb][:], P, f"wqs{cb}", cb,
                              "dve" if cb == 0 else "act")
                         for cb in range(2)]
                wkt_s = [fold(wbig_sb[cb][:, WK0:WK0 + C], C, f"wks{cb}", cb,
                              "dve" if cb == 0 else "act")
                         for cb in range(2)]
                wvt_s = [fold(wbig_sb[cb][:, WV0:WV0 + C], C, f"wvs{cb}", cb,
                              "dve" if cb == 0 else "act")
                         for cb in range(2)]

                junk(0, 3)

                # bias folds: b' = b + W @ bias_c = b + W_s @ bias2
                bq2 = quad()
                for cob in range(2):
                    for s, wt in ((0, wkt_s), (1, wvt_s)):
                        psl = bq2[:, 2 * s + cob, 0:1]
                        nc.tensor.matmul(psl, wt[0][:, cob * P:(cob + 1) * P],
                                         bias2_bf[0][:], start=True,
                                         stop=False)
                        nc.tensor.matmul(psl, wt[1][:, cob * P:(cob + 1) * P],
                                         bias2_bf[1][:], start=False,
                                         stop=True)
                bq3 = quad()
                nc.tensor.matmul(bq3[0:1, 0, 0:P], bias2_bf[0][:], wqt_s[0][:],
                                 start=True, stop=False)
                nc.tensor.matmul(bq3[0:1, 0, 0:P], bias2_bf[1][:], wqt_s[1][:],
                                 start=False, stop=True)

                bkp, bvp = [], []
                for cob in range(2):
                    t = small.tile([P, 1], f32, tag="bkp", name="bkp")
                    nc.vector.tensor_add(t[:], bq2[:, cob, 0:1],
                                         wsml(cob, BK0))
                    bkp.append(t)
                    t = small.tile([P, 1], f32, tag="bvp", name="bvp")
                    nc.vector.tensor_add(t[:], bq2[:, 2 + cob, 0:1],
                                         wsml(cob, BV0))
                    bvp.append(t)
                bq_row = small.tile([1, P], f32, tag="bqp", name="bqp")
                nc.vector.tensor_add(bq_row[:], bq3[0:1, 0, 0:P],
                                     bq_row_bf[:])
                bq_bc = persist.tile([P, P], f32, tag="bqbc", name="bqbc")
                nc.gpsimd.partition_broadcast(bq_bc[:], bq_row[:])

                junk(1, 3)

                # ---------- k, v convs (K=256 via 2 bf16 matmuls) --------
                def conv_full(wt, b_sb, dst):
                    for cob in range(2):
                        for qd in range(2):
                            ps = quad()
                            for s in range(4):
                                ch = 4 * qd + s
                                sl = slice(ch * CHUNK, (ch + 1) * CHUNK)
                                nc.tensor.matmul(
                                    ps[:, s, :],
                                    wt[0][:, cob * P:(cob + 1) * P],
                                    x_sb[0][:, sl], start=True, stop=False)
                                nc.tensor.matmul(
                                    ps[:, s, :],
                                    wt[1][:, cob * P:(cob + 1) * P],
                                    x_sb[1][:, sl], start=False, stop=True)
                            for s in range(4):
                                ch = 4 * qd + s
                                sl = slice(ch * CHUNK, (ch + 1) * CHUNK)
                                if s % 2 == 0:
                                    nc.vector.tensor_scalar_add(
                                        dst(cob, sl), ps[:, s, :],
                                        b_sb[cob][:])
                                else:
                                    nc.scalar.activation(
                                        dst(cob, sl), ps[:, s, :],
                                        AF.Identity, bias=b_sb[cob][:])

                conv_full(wkt_s, bkp, lambda cob, sl: k_f8[:, cob, sl])
                conv_full(wvt_s, bvp, lambda cob, sl: v_f8[cob][:, sl])

                # v_att[j, c] = v[j//16, 256*(j%16)+c]; [j', pair, jlo, c]
                # so a [128, 2, 128] DoubleRow stationary covers two
                # j-blocks. Spread over three DMA queues.
                for jb in range(NB):
                    cb = jb // 16
                    p0 = (jb % 16) * 8
                    src = v_f8[cb][p0:p0 + 8, :].rearrange(
                        "p (r c) -> p r c", c=C)
                    eng = (nc.sync, nc.scalar, nc.gpsimd)[jb % 3]
                    eng.dma_start(v_att[:, jb // 2, jb % 2, :], src)

                # ---------- qT: q_att.T for this core's half -------------
                # qT[m, cb, 16a+r] = qconv[a, 256r+128cb+m]; bias bq'[a] is
                # added during the copies via the partition-broadcast row.
                # All 32 matmul pairs of a quad are emitted before its
                # copies so no write-after-read chain forms on the quad.
                it = [(r, cb) for r in range(16) for cb in range(2)]
                for qd in range(2):
                    ps = quad()
                    for k16 in range(16):
                        r, cb = it[qd * 16 + k16]
                        sl = slice(256 * r + cb * P, 256 * r + (cb + 1) * P)
                        psl = ps[:, k16 // 4, (k16 % 4) * P:(k16 % 4 + 1) * P]
                        nc.tensor.matmul(psl, x_sb[0][:, sl], wqt_s[0][:],
                                         start=True, stop=False)
                        nc.tensor.matmul(psl, x_sb[1][:, sl], wqt_s[1][:],
                                         start=False, stop=True)
                    for k16 in range(16):
                        r, cb = it[qd * 16 + k16]
                        psl = ps[:, k16 // 4, (k16 % 4) * P:(k16 % 4 + 1) * P]
                        nc.vector.scalar_tensor_tensor(
                            qTv[:, cb, :, r], psl, 1.0, bq_bc[:],
                            op0=OP.mult, op1=OP.add)

            # ---------- attention + projection ----------
            with (
                tc.tile_pool(name="sqp", bufs=2, space="PSUM") as sqp,
                tc.tile_pool(name="hacc", bufs=2, space="PSUM") as hacc,
                tc.tile_pool(name="lacc", bufs=1, space="PSUM") as lacc,
                tc.tile_pool(name="misc", bufs=1, space="PSUM") as miscp,
            ):
                def make_stage5a(isl, hps, lp):
                    """Drain the accumulators right at chunk end so their
                    PSUM banks recycle before the next chunk's accums; the
                    denominator row is bounced through SBUF to DRAM."""
                    l_sb = small.tile([1, CHUNK], f32, tag="lsb", name="lsb")
                    nc.vector.tensor_copy(l_sb[:], lp[:])
                    nc.sync.dma_start(lout_ap[0:1, isl], l_sb[:])
                    hT = [htpool.tile([P, CHUNK], bf16, tag="hT", name="hT")
                          for _ in range(2)]
                    nc.vector.tensor_copy(hT[0][:], hps[0][:])
                    nc.vector.tensor_copy(hT[1][:], hps[1][:])
                    return hT

                def make_stage5b(isl, hT):
                    def stage5b():
                        for cob in range(2):
                            pp = miscp.tile([P, CHUNK], f32, tag="pp",
                                            name="pp")
                            nc.tensor.matmul(
                                pp[:], wpt_bf[0][:, cob * P:(cob + 1) * P],
                                hT[0][:], start=True, stop=False)
                            nc.tensor.matmul(
                                pp[:], wpt_bf[1][:, cob * P:(cob + 1) * P],
                                hT[1][:], start=False, stop=True)
                            o_t = opool.tile([P, CHUNK], bf16, tag="ot",
                                             name="ot")
                            nc.vector.tensor_copy(o_t[:], pp[:])
                            nc.sync.dma_start(
                                out_ap[cob * P:(cob + 1) * P, isl], o_t[:])
                    return stage5b

                pending5 = None
                for sc in range(SC):
                    isl = slice(sc * CHUNK, (sc + 1) * CHUNK)
                    hps = None
                    lp = None

                    def accums(pair, e_pair):
                        """h_attT and softmax-denominator accumulation for
                        a pair of j-blocks (DoubleRow, K=256)."""
                        st, sp = (pair == 0), (pair == NB // 2 - 1)
                        nc.tensor.matmul(hps[0][:], v_att[:, pair, :, 0:P],
                                         e_pair[:], start=st, stop=sp,
                                         perf_mode=DR)
                        nc.tensor.matmul(hps[1][:], v_att[:, pair, :, P:C],
                                         e_pair[:], start=st, stop=sp,
                                         perf_mode=DR)
                        nc.tensor.matmul(lp[:], ones_pair_f8[:, :, 0:1],
                                         e_pair[:], start=st, stop=sp,
                                         perf_mode=DR)

                    prev = None
                    for m in range(NB // 2):
                        ps_s = sqp.tile([P, 2, CHUNK], f32, tag="sq",
                                        name="sq")
                        for s in range(2):
                            jb = 2 * m + s
                            jsl = slice(jb * P, (jb + 1) * P)
                            nc.tensor.matmul(ps_s[:, s, :], k_f8[:, :, jsl],
                                             qT[:, :, isl], start=True,
                                             stop=True, perf_mode=DR)
                        e_pair = epool.tile([P, 2, CHUNK], f8, tag="e",
                                            name="e")
                        # e^{s/16 - 4}: the -4 keeps E in fp8's finite range
                        # and cancels exactly in the softmax normalization
                        nc.scalar.activation(e_pair[:], ps_s[:], AF.Exp,
                                             scale=float(C) ** -0.5,
                                             bias=neg4_sb[:])
                        if m == 1 and pending5 is not None:
                            pending5()
                            pending5 = None
                        if prev is not None:
                            if hps is None:
                                hps = [hacc.tile([P, CHUNK], f32, tag="hacc",
                                                 name="hacc")
                                       for _ in range(2)]
                                lp = lacc.tile([1, CHUNK], f32, tag="lacc",
                                               name="lacc")
                            accums(*prev)
                        prev = (m, e_pair)
                    accums(*prev)
                    hT = make_stage5a(isl, hps, lp)
                    pending5 = make_stage5b(isl, hT)
                pending5()

    nc.compile()
    return nc


def _get_exec():
    if "fn" in _cache:
        return _cache["fn"], _cache["zfn"], _cache["in_names"]

    import jax
    import jax.numpy as jnp
    import ml_dtypes
    from jax.experimental.shard_map import shard_map
    from jax.sharding import Mesh, NamedSharding, PartitionSpec as PS

    from concourse import bass2jax, mybir

    try:
        jax.config.update("jax_compilation_cache_dir", "/tmp/jax_cc_cache")
        jax.config.update("jax_persistent_cache_min_compile_time_secs", 0.0)
    except Exception:
        pass

    nc = _build_nc()
    _cache["nc"] = nc
    bass2jax.install_neuronx_cc_hook()

    partition_name = (nc.partition_id_tensor.name
                      if nc.partition_id_tensor else None)
    in_names, out_names, out_avals = [], [], []
    for alloc in nc.m.functions[0].allocations:
        if not isinstance(alloc, mybir.MemoryLocationSet):
            continue
        name = alloc.memorylocations[0].name
        if alloc.kind == "ExternalInput":
            if name != partition_name:
                in_names.append(name)
        elif alloc.kind == "ExternalOutput":
            out_avals.append(jax.core.ShapedArray(
                tuple(alloc.tensor_shape), mybir.dt.np(alloc.dtype)))
            out_names.append(name)
    n_params = len(in_names)
    all_in_names = in_names + out_names
    if partition_name:
        all_in_names = all_in_names + [partition_name]

    def _body(*args):
        operands = list(args)
        if partition_name:
            operands.append(bass2jax.partition_id_tensor())
        outs = bass2jax._bass_exec_p.bind(
            *operands, out_avals=tuple(out_avals),
            in_names=tuple(all_in_names), out_names=tuple(out_names),
            lowering_input_output_aliases=(), sim_require_finite=True,
            sim_require_nnan=True, nc=nc)
        return tuple(outs)

    devices = np.asarray(jax.devices()[:NCORES]).reshape(B, 2)
    mesh = Mesh(devices, ("pair", "half"))
    spec_by_name = {"x": PS("pair"), "qpack": PS("half"), "wbig": PS(),
                    "wsml": PS(), "windt": PS()}
    in_specs = (tuple(spec_by_name[n] for n in in_names)
                + (PS(("pair", "half")), PS(("pair", "half"))))
    out_specs = (PS(("pair", "half")), PS(("pair", "half")))

    fn = jax.jit(
        shard_map(_body, mesh=mesh, in_specs=in_specs,
                  out_specs=out_specs, check_rep=False),
        donate_argnums=(n_params, n_params + 1), keep_unused=True)

    zsharding = NamedSharding(mesh, PS(("pair", "half")))
    zfn = jax.jit(
        lambda: (jnp.zeros((NCORES * C, HALF), ml_dtypes.bfloat16),
                 jnp.zeros((NCORES, HALF), np.float32)),
        out_shardings=(zsharding, zsharding))

    _cache["fn"] = fn
    _cache["zfn"] = zfn
    _cache["in_names"] = in_names
    return fn, zfn, in_names


def _pack_inputs(x, gn_w, gn_b, wq, bq, wk, bk, wv, bv, wp, bp):
    import ml_dtypes
    bfd = ml_dtypes.bfloat16
    f = np.float32
    asrt = lambda a: np.asarray(a, f)
    x = np.asarray(x, f).reshape(B * C, N)
    x_bf = x.astype(bfd)

    wq, wk, wv, wp = asrt(wq), asrt(wk), asrt(wv), asrt(wp)
    bq, bk, bv, bp = asrt(bq), asrt(bk), asrt(bv), asrt(bp)
    gn_w, gn_b = asrt(gn_w), asrt(gn_b)

    qpack = np.empty((2 * (C + 1), P), bfd)
    wqT = wq.T.astype(bfd)
    for j in range(2):
        qpack[j * (C + 1):j * (C + 1) + C] = wqT[:, j * P:(j + 1) * P]
        qpack[j * (C + 1) + C] = bq[j * P:(j + 1) * P].astype(bfd)

    wbig = np.empty((C, WBCOLS), bfd)
    wbig[:, WK0:WK0 + C] = wk.T.astype(bfd)
    wbig[:, WV0:WV0 + C] = wv.T.astype(bfd)
    wbig[:, WP0:WP0 + C] = wp.T.astype(bfd)

    wsml = np.zeros((C, WSCOLS), f)
    wsml[:, BK0] = bk
    wsml[:, BV0] = bv
    wsml[:, BP0] = bp
    wsml[:, GW0] = gn_w
    wsml[:, GB0] = gn_b
    ind = np.zeros((C, GROUPS), f)
    ind[np.arange(C), np.arange(C) // GSIZE] = 1.0
    wsml[:, IND0:IND0 + GROUPS] = ind
    windt = np.ascontiguousarray(ind.T)
    return x, x_bf, qpack, wbig, wsml, windt, bp


def kernel(x, gn_w, gn_b, wq, bq, wk, bk, wv, bv, wp, bp):
    fn, zfn, in_names = _get_exec()
    x_f32, x_bf, qpack, wbig, wsml, windt, bp_f = _pack_inputs(
        x, gn_w, gn_b, wq, bq, wk, bk, wv, bv, wp, bp)
    arrs = {"x": x_bf, "qpack": qpack, "wbig": wbig, "wsml": wsml,
            "windt": windt}
    p_out, l_out = fn(*(arrs[n] for n in in_names), *zfn())
    # p_out: (8*C, HALF) bf16 unnormalized, l_out: (8, HALF) f32,
    # blocks ordered core = 2b + j; host applies p/l + bp + residual.
    p = np.asarray(p_out).astype(np.float32).reshape(B, 2, C, HALF)
    l = np.asarray(l_out).astype(np.float32).reshape(B, 2, 1, HALF)
    p = p / l + bp_f[None, None, :, None]
    out = np.empty((B, C, N), np.float32)
    for j in range(2):
        out[:, :, j * HALF:(j + 1) * HALF] = p[:, j]
    out += x_f32.reshape(B, C, N)
    return out.reshape(B, C, H, W)


# revision 8
# speedup vs baseline: 4863.2021x; 4863.2021x over previous
"""AttnBlock on 8 Trainium2 NeuronCores via Bass/Tile.

Reference computation (shapes hardcoded): x (4, 256, 64, 64) f32,
GroupNorm(32 groups) -> q/k/v 1x1 conv -> HWxHW attention (with the
reference's raw-view reshape (C,N)->(N,C) for q and v) -> proj -> x + p.

Sharding: 8 cores = 4 batch elements x 2 query-halves, mesh (pair=4,
half=2). Core (b, j) handles batch b and attention rows n in
[j*2048, (j+1)*2048). The raw view means q_att rows [j*2048,(j+1)*2048)
depend only on wq rows [j*128,(j+1)*128), so each core computes: full
k/v, its half of qT, its half of the attention, and p columns
[j*2048,(j+1)*2048). No collectives.

GroupNorm is FOLDED into the q/k/v conv weights: h = scale_c*x + bias_c
per channel, so W@h = (W*scale)@x + W_s@(bias_c/scale). The per-channel
scale rides the weight converts (same op count as a plain convert), the
bias terms become tiny matmuls, and h is never materialized - the convs
consume x directly.

Key layout identity (N=4096=HW, C=256): q_att[n,c] = q[n//16, 256*(n%16)+c],
so  qT[c, 16a+r] = (x[:, 256r:256r+256].T @ wq_half_s.T)[c, a].
qT is stored COLUMN-PERMUTED as qT2[c, cb, 128r+a] so each PSUM->SBUF
copy lands contiguously; one scalar_tensor_tensor per (quad, cb) casts
8 matmul outputs at once while adding the folded q bias via a
partition-broadcast row. The attention then simply runs on permuted i
columns (chunk sc covers r in [4sc,4sc+4)) and the host un-permutes the
output columns during the unshard.

Attention is computed transposed: ST[j,i] = sum_c k[c,j]*qT[c,i], then
E = exp(ST/16 - 4) (scores are ~N(0,1): no max subtraction needed, and
the -4 centers E in fp8 range and cancels in the normalization),
h_attT[c,i] = sum_j v_att[j,c]*E[j,i] accumulated in PSUM over j-block
pairs. k/qT/E/v_att are fp8(e4m3) with K=256 packed [128,2,.] for
DoubleRow matmuls. The softmax denominator accumulates via M=1
ones-matmuls and ships to the host as a second tiny output; the host
performs p/l + bp during the unshard, so the device ships the
unnormalized wp @ h_attT in bf16 (bf16 is floating, so the larger
magnitude costs no relative precision). In the attention phase ACT does
nothing but the 64 exps (the exp chain is the co-bottleneck with the
PE's matmul stream), all PSUM->SBUF traffic runs on DVE, and the pair
pipeline is FLAT across chunk boundaries - the accumulation matmuls run
one pair behind the score matmuls everywhere, so the PE never idles at
a chunk edge waiting for exp.

Host I/O is minimized: x ships bf16, the big weights ship bf16
(wk|wv|wp transposed), one-hots f32, per-half q weights per half.
Outputs are the bf16 p-halves plus the f32 denominator rows.
"""

import numpy as np

B, C, H, W = 4, 256, 64, 64
N = H * W            # 4096 pixels
HALF = N // 2        # 2048 attention rows per core
GROUPS = 32
GSIZE = C // GROUPS  # 8 channels per group
EPS = 1e-5
NCORES = 8
P = 128              # partitions
NB = N // P          # 32 j-blocks of 128
SC = 4               # i super-chunks per core
CHUNK = 512          # i columns per chunk (one PSUM bank)
NPAIR = NB // 2      # 16 j-block pairs per chunk

# wbig column layout (bf16, C rows): wk.T | wv.T | wp.T
WK0, WV0, WP0 = 0, C, 2 * C
WBCOLS = 3 * C
# wsml column layout (f32, C rows)
BK0, BV0, BP0, GW0, GB0 = 0, 1, 2, 3, 4
IND0 = 5
WSCOLS = IND0 + GROUPS

_cache = {}


def _build_nc():
    import concourse.tile as tile
    from concourse import bacc, mybir

    f32 = mybir.dt.float32
    bf16 = mybir.dt.bfloat16
    f8 = mybir.dt.float8e4
    AF = mybir.ActivationFunctionType
    OP = mybir.AluOpType

    nc = bacc.Bacc("TRN2", target_bir_lowering=False, debug=False,
                   num_devices=NCORES)

    x_ap = nc.dram_tensor("x", [C, N], bf16, kind="ExternalInput").ap()
    qpack_ap = nc.dram_tensor("qpack", [C + 1, P], bf16,
                              kind="ExternalInput").ap()
    wbig_ap = nc.dram_tensor("wbig", [C, WBCOLS], bf16,
                             kind="ExternalInput").ap()
    wsml_ap = nc.dram_tensor("wsml", [C, WSCOLS], f32,
                             kind="ExternalInput").ap()
    windt_ap = nc.dram_tensor("windt", [GROUPS, C], f32,
                              kind="ExternalInput").ap()
    out_ap = nc.dram_tensor("out", [C, HALF], bf16, kind="ExternalOutput").ap()
    lout_ap = nc.dram_tensor("lout", [1, HALF], f32,
                             kind="ExternalOutput").ap()

    with tile.TileContext(nc) as tc:
        with (
            tc.tile_pool(name="persist", bufs=1) as persist,
            tc.tile_pool(name="small", bufs=4) as small,
            tc.tile_pool(name="epool", bufs=4) as epool,
            tc.tile_pool(name="htpool", bufs=4) as htpool,
            tc.tile_pool(name="opool", bufs=4) as opool,
        ):
            # constants first so nothing queues behind the big DMAs
            warm_w = persist.tile([P, P], bf16, tag="warmw", name="warmw")
            nc.vector.memset(warm_w[:], 1.0)
            ones_pair_f8 = persist.tile([P, 2, 16], f8, tag="ones_pair",
                                        name="ones_pair")
            nc.vector.memset(ones_pair_f8[:], 1.0)
            eps_sb = persist.tile([GROUPS, 1], f32, tag="eps", name="eps")
            nc.vector.memset(eps_sb[:], EPS)
            neg4_sb = persist.tile([P, 1], f32, tag="neg4", name="neg4")
            nc.vector.memset(neg4_sb[:], -4.0)

            # ---------- x + weights across the three DMA queues ----------
            # Bigger chunks (4KB per-partition lines) use the per-queue DMA
            # bandwidth better; the per-queue byte budget is balanced so
            # all of x lands at roughly the same time on every queue.
            x_sb = [persist.tile([P, N], bf16, tag=f"x{cb}", name=f"x{cb}")
                    for cb in range(2)]

            def xdma(eng, cb, c0, c1):
                eng.dma_start(x_sb[cb][:, c0:c1],
                              x_ap[cb * P:(cb + 1) * P, c0:c1])

            xdma(nc.sync, 0, 0, 2048)
            xdma(nc.scalar, 1, 0, 2048)
            xdma(nc.gpsimd, 0, 2048, 3072)
            xdma(nc.gpsimd, 1, 2048, 3072)
            xdma(nc.sync, 1, 3072, 4096)
            xdma(nc.scalar, 0, 3072, 4096)

            def rows(cb):
                return slice(cb * P, (cb + 1) * P)

            wbig_sb, qp_sb = [], []
            for cb in range(2):
                t = persist.tile([P, WBCOLS], bf16, tag=f"wb{cb}",
                                 name=f"wb{cb}")
                nc.gpsimd.dma_start(t[:], wbig_ap[rows(cb), :])
                wbig_sb.append(t)
                t = persist.tile([P, P], bf16, tag=f"qp{cb}", name=f"qp{cb}")
                nc.sync.dma_start(t[:], qpack_ap[rows(cb), :])
                qp_sb.append(t)
            bq_row_bf = persist.tile([1, P], bf16, tag="bqrow", name="bqrow")
            nc.sync.dma_start(bq_row_bf[:], qpack_ap[C:C + 1, :])
            wsml_sb = []
            for cb in range(2):
                t = persist.tile([P, WSCOLS], f32, tag=f"ws{cb}",
                                 name=f"ws{cb}")
                nc.scalar.dma_start(t[:], wsml_ap[rows(cb), :])
                wsml_sb.append(t)
            windt_sb = persist.tile([GROUPS, C], f32, tag="windt",
                                    name="windt")
            nc.scalar.dma_start(windt_sb[:], windt_ap[:, :])

            def wsml(cb, c0, c1=None):
                c1 = c0 + 1 if c1 is None else c1
                return wsml_sb[cb][:, c0:c1]

            wpt_bf = [wbig_sb[cb][:, WP0:WP0 + C] for cb in range(2)]

            DR = mybir.MatmulPerfMode.DoubleRow

            # persistent fp8 tensors
            k_f8 = persist.tile([P, 2, N], f8, tag="kf8", name="kf8")
            v_f8 = [persist.tile([P, N], f8, tag=f"vf8{cb}", name=f"vf8{cb}")
                    for cb in range(2)]
            v_att = persist.tile([P, NB // 2, 2, C], f8, tag="vatt",
                                 name="vatt")
            # permuted q_att.T: qT2[c', cb, 128r+a] = q_att.T[128cb+c', 16a+r]
            qT2 = persist.tile([P, 2, HALF], f8, tag="qT", name="qT")

            # ---------- pre-attention PSUM: two [P,4,512] quads ----------
            with tc.tile_pool(name="ps4", bufs=2, space="PSUM") as ps4:
                def quad():
                    return ps4.tile([P, 4, CHUNK], f32, tag="q4", name="q4")

                # dummy matmuls consuming each x chunk as it arrives keep
                # the HAM clock-gate at full rate into the convs
                def junk(cb, ch):
                    jq = quad()
                    nc.tensor.matmul(
                        jq[:, 0, :], warm_w[:],
                        x_sb[cb][:, ch * CHUNK:(ch + 1) * CHUNK],
                        start=True, stop=True)

                for rep in range(2):
                    for q in range(4):
                        for cb in range(2):
                            junk(cb, 2 * q + rep)

                # ---------- GroupNorm stats ----------
                m1m2 = []
                for cb in range(2):
                    xv = x_sb[cb].rearrange("p (s q) -> p s q", q=512)
                    stats = small.tile([P, 8, 6], f32, tag="bnstats",
                                       name="bnstats")
                    for s in range(8):
                        nc.vector.bn_stats(stats[:, s, :], xv[:, s, :])
                    mv = small.tile([P, 2], f32, tag="bnmv", name="bnmv")
                    nc.vector.bn_aggr(mv[:], stats[:])
                    mm12 = small.tile([P, 2], f32, tag="m1m2", name="m1m2")
                    nc.vector.tensor_copy(mm12[:, 0:1], mv[:, 0:1])
                    # E[x^2] = mean^2 + var in one fused op
                    nc.vector.scalar_tensor_tensor(
                        mm12[:, 1:2], mv[:, 0:1], mv[:, 0:1], mv[:, 1:2],
                        op0=OP.mult, op1=OP.add)
                    m1m2.append(mm12)

                gq = quad()
                # group sums: [32, 2] = sum over channels in group
                nc.tensor.matmul(gq[0:GROUPS, 0, 0:2],
                                 wsml(0, IND0, IND0 + GROUPS),
                                 m1m2[0][:], start=True, stop=False)
                nc.tensor.matmul(gq[0:GROUPS, 0, 0:2],
                                 wsml(1, IND0, IND0 + GROUPS),
                                 m1m2[1][:], start=False, stop=True)

                gstats = small.tile([GROUPS, 2], f32, tag="gstats",
                                    name="gstats")
                nc.vector.tensor_scalar_mul(gstats[:, 0:1],
                                            gq[0:GROUPS, 0, 0:1],
                                            1.0 / GSIZE)
                musq = small.tile([GROUPS, 1], f32, tag="gmusq", name="gmusq")
                nc.vector.tensor_mul(musq[:], gstats[:, 0:1], gstats[:, 0:1])
                # var = E[x^2]/8 - mean^2 in one fused op
                gvar = small.tile([GROUPS, 1], f32, tag="gvar", name="gvar")
                nc.vector.scalar_tensor_tensor(
                    gvar[:], gq[0:GROUPS, 0, 1:2], 1.0 / GSIZE, musq[:],
                    op0=OP.mult, op1=OP.subtract)
                gsd = small.tile([GROUPS, 1], f32, tag="gsd", name="gsd")
                nc.scalar.activation(gsd[:], gvar[:], AF.Sqrt, bias=eps_sb[:])
                nc.vector.reciprocal(gstats[:, 1:2], gsd[:])

                junk(0, 1)

                # scatter group stats to channels; per-channel fold params
                scale_c, bias2_bf = [], []
                sq2 = quad()
                for cb in range(2):
                    nc.tensor.matmul(sq2[:, cb, 0:2],
                                     windt_sb[:, cb * P:(cb + 1) * P],
                                     gstats[:], start=True, stop=True)
                    sc_ = small.tile([P, 1], f32, tag="scalec", name="scalec")
                    nc.vector.tensor_mul(sc_[:], sq2[:, cb, 1:2],
                                         wsml(cb, GW0))
                    rsc = small.tile([P, 1], f32, tag="rsc", name="rsc")
                    nc.vector.reciprocal(rsc[:], sc_[:])
                    # bias2 = bias_c/scale = gnb/scale - mean, fused; the
                    # folded weights then give W_s @ bias2 = W @ bias_c
                    b2b = small.tile([P, 1], bf16, tag="b2b", name="b2b")
                    nc.vector.scalar_tensor_tensor(
                        b2b[:], wsml(cb, GB0), rsc[:], sq2[:, cb, 0:1],
                        op0=OP.mult, op1=OP.subtract)
                    scale_c.append(sc_)
                    bias2_bf.append(b2b)

                junk(1, 1)

                # folded bf16 weights: W_s = W.T * scale_c (per partition)
                def fold(src_ap, cols, tag, cb, eng):
                    t = persist.tile([P, cols], bf16, tag=tag, name=tag)
                    if eng == "dve":
                        nc.vector.tensor_scalar_mul(t[:], src_ap,
                                                    scale_c[cb][:])
                    else:
                        nc.scalar.activation(t[:], src_ap, AF.Identity,
                                             scale=scale_c[cb][:])
                    return t

                wqt_s = [fold(qp_sb[cb][:], P, f"wqs{cb}", cb,
                              "dve" if cb == 0 else "act")
                         for cb in range(2)]
                wkt_s = [fold(wbig_sb[cb][:, WK0:WK0 + C], C, f"wks{cb}", cb,
                              "dve" if cb == 0 else "act")
                         for cb in range(2)]
                wvt_s = [fold(wbig_sb[cb][:, WV0:WV0 + C], C, f"wvs{cb}", cb,
                              "dve" if cb == 0 else "act")
                         for cb in range(2)]

                junk(0, 3)

                # bias folds: b' = b + W @ bias_c = b + W_s @ bias2
                bq2 = quad()
                for cob in range(2):
                    for s, wt in ((0, wkt_s), (1, wvt_s)):
                        psl = bq2[:, 2 * s + cob, 0:1]
                        nc.tensor.matmul(psl, wt[0][:, cob * P:(cob + 1) * P],
                                         bias2_bf[0][:], start=True,
                                         stop=False)
                        nc.tensor.matmul(psl, wt[1][:, cob * P:(cob + 1) * P],
                                         bias2_bf[1][:], start=False,
                                         stop=True)
                bq3 = quad()
                nc.tensor.matmul(bq3[0:1, 0, 0:P], bias2_bf[0][:], wqt_s[0][:],
                                 start=True, stop=False)
                nc.tensor.matmul(bq3[0:1, 0, 0:P], bias2_bf[1][:], wqt_s[1][:],
                                 start=False, stop=True)

                bkp, bvp = [], []
                for cob in range(2):
                    t = small.tile([P, 1], f32, tag="bkp", name="bkp")
                    nc.vector.tensor_add(t[:], bq2[:, cob, 0:1],
                                         wsml(cob, BK0))
                    bkp.append(t)
                    t = small.tile([P, 1], f32, tag="bvp", name="bvp")
                    nc.vector.tensor_add(t[:], bq2[:, 2 + cob, 0:1],
                                         wsml(cob, BV0))
                    bvp.append(t)
                # bq' row replicated x8 then partition-broadcast, so one
                # STT per (quad, cb) adds the bias to 8 slots at once
                bq_row8 = small.tile([1, 8, P], f32, tag="bqp", name="bqp")
                for rep in range(8):
                    nc.vector.tensor_add(bq_row8[:, rep, :],
                                         bq3[0:1, 0, 0:P], bq_row_bf[:])
                bq_bc8 = persist.tile([P, 8 * P], f32, tag="bqbc",
                                      name="bqbc")
                nc.gpsimd.partition_broadcast(
                    bq_bc8[:], bq_row8.rearrange("o r p -> o (r p)"))

                junk(1, 3)

                # ---------- k, v convs (K=256 via 2 bf16 matmuls) --------
                def conv_full(wt, b_sb, dst):
                    for cob in range(2):
                        for qd in range(2):
                            ps = quad()
                            for s in range(4):
                                ch = 4 * qd + s
                                sl = slice(ch * CHUNK, (ch + 1) * CHUNK)
                                nc.tensor.matmul(
                                    ps[:, s, :],
                                    wt[0][:, cob * P:(cob + 1) * P],
                                    x_sb[0][:, sl], start=True, stop=False)
                                nc.tensor.matmul(
                                    ps[:, s, :],
                                    wt[1][:, cob * P:(cob + 1) * P],
                                    x_sb[1][:, sl], start=False, stop=True)
                            for h2 in range(2):
                                sl = slice((4 * qd + 2 * h2) * CHUNK,
                                           (4 * qd + 2 * h2 + 2) * CHUNK)
                                if h2 == 0:
                                    nc.vector.tensor_scalar_add(
                                        dst(cob, sl), ps[:, 0:2, :],
                                        b_sb[cob][:])
                                else:
                                    nc.scalar.activation(
                                        dst(cob, sl), ps[:, 2:4, :],
                                        AF.Identity, bias=b_sb[cob][:])

                conv_full(wkt_s, bkp, lambda cob, sl: k_f8[:, cob, sl])
                conv_full(wvt_s, bvp, lambda cob, sl: v_f8[cob][:, sl])

                # v_att[j, c] = v[j//16, 256*(j%16)+c]; [j', pair, jlo, c]
                # so a [128, 2, 128] DoubleRow stationary covers two
                # j-blocks. Spread over three DMA queues.
                for jb in range(NB):
                    cb = jb // 16
                    p0 = (jb % 16) * 8
                    src = v_f8[cb][p0:p0 + 8, :].rearrange(
                        "p (r c) -> p r c", c=C)
                    eng = (nc.sync, nc.scalar, nc.gpsimd)[jb % 3]
                    eng.dma_start(v_att[:, jb // 2, jb % 2, :], src)

                # ---------- qT2: permuted q_att.T for this core's half ---
                # qT2[m, cb, 128r+a] = qconv[a, 256r+128cb+m]. All 32
                # matmul pairs of a quad are emitted before its two big
                # copies so no write-after-read chain forms on the quad;
                # each STT casts 8 slots and adds the bias in one op.
                for qd in range(2):
                    ps = quad()
                    for cb in range(2):
                        for k8 in range(8):
                            r = 8 * qd + k8
                            s16 = 8 * cb + k8
                            sl = slice(256 * r + cb * P,
                                       256 * r + (cb + 1) * P)
                            psl = ps[:, s16 // 4,
                                     (s16 % 4) * P:(s16 % 4 + 1) * P]
                            nc.tensor.matmul(psl, x_sb[0][:, sl], wqt_s[0][:],
                                             start=True, stop=False)
                            nc.tensor.matmul(psl, x_sb[1][:, sl], wqt_s[1][:],
                                             start=False, stop=True)
                    for cb in range(2):
                        nc.vector.scalar_tensor_tensor(
                            qT2[:, cb, qd * 1024:(qd + 1) * 1024],
                            ps[:, 2 * cb:2 * cb + 2, :].rearrange(
                                "p s q -> p (s q)"),
                            1.0, bq_bc8[:], op0=OP.mult, op1=OP.add)

            # ---------- attention + projection ----------
            # flat pair pipeline across all 4 chunks: scores(p) -> exp(p)
            # -> accums(p-1); chunk bookkeeping (accumulator drain, wp
            # projection, output DMA) is spliced into the stream so the
            # PE and ACT never wait at a chunk boundary.
            with (
                tc.tile_pool(name="sqp", bufs=2, space="PSUM") as sqp,
                tc.tile_pool(name="hacc", bufs=3, space="PSUM") as hacc,
                tc.tile_pool(name="lacc", bufs=1, space="PSUM") as lacc,
            ):
                def make_stage5a(isl, hps, lp):
                    """Drain the accumulators right after their last matmul
                    so the PSUM banks recycle before the next chunk's
                    accums; the denominator row bounces through SBUF."""
                    l_sb = small.tile([1, CHUNK], f32, tag="lsb", name="lsb")
                    nc.vector.tensor_copy(l_sb[:], lp[:])
                    nc.sync.dma_start(lout_ap[0:1, isl], l_sb[:])
                    hT = [htpool.tile([P, CHUNK], bf16, tag="hT", name="hT")
                          for _ in range(2)]
                    nc.vector.tensor_copy(hT[0][:], hps[0][:])
                    nc.vector.tensor_copy(hT[1][:], hps[1][:])
                    return hT

                def make_stage5b(isl, hT):
                    def stage5b():
                        # the projection borrows one slot of the scores
                        # ring (same tag/shape) instead of its own bank
                        pp = sqp.tile([P, 2, CHUNK], f32, tag="sq",
                                      name="sq")
                        for cob in range(2):
                            nc.tensor.matmul(
                                pp[:, cob, :],
                                wpt_bf[0][:, cob * P:(cob + 1) * P],
                                hT[0][:], start=True, stop=False)
                            nc.tensor.matmul(
                                pp[:, cob, :],
                                wpt_bf[1][:, cob * P:(cob + 1) * P],
                                hT[1][:], start=False, stop=True)
                            o_t = opool.tile([P, CHUNK], bf16, tag="ot",
                                             name="ot")
                            nc.vector.tensor_copy(o_t[:], pp[:, cob, :])
                            nc.sync.dma_start(
                                out_ap[cob * P:(cob + 1) * P, isl], o_t[:])
                    return stage5b

                state = {"hps": None, "lp": None}
                pending5 = [None]

                def accums(gp, e_pair):
                    """h_attT and softmax-denominator accumulation for one
                    j-block pair of chunk gp//NPAIR (DoubleRow, K=256)."""
                    pair = gp % NPAIR
                    st, sp = (pair == 0), (pair == NPAIR - 1)
                    if st:
                        state["hps"] = [
                            hacc.tile([P, CHUNK], f32, tag="hacc",
                                      name="hacc") for _ in range(2)]
                        state["lp"] = lacc.tile([1, CHUNK], f32, tag="lacc",
                                                name="lacc")
                    hps, lp = state["hps"], state["lp"]
                    nc.tensor.matmul(hps[0][:], v_att[:, pair, :, 0:P],
                                     e_pair[:], start=st, stop=sp,
                                     perf_mode=DR)
                    nc.tensor.matmul(hps[1][:], v_att[:, pair, :, P:C],
                                     e_pair[:], start=st, stop=sp,
                                     perf_mode=DR)
                    nc.tensor.matmul(lp[:], ones_pair_f8[:, :, 0:1],
                                     e_pair[:], start=st, stop=sp,
                                     perf_mode=DR)
                    if sp:
                        sc = gp // NPAIR
                        isl = slice(sc * CHUNK, (sc + 1) * CHUNK)
                        hT = make_stage5a(isl, hps, lp)
                        pending5[0] = make_stage5b(isl, hT)

                prev = None
                for gp in range(SC * NPAIR):
                    sc, m = divmod(gp, NPAIR)
                    isl = slice(sc * CHUNK, (sc + 1) * CHUNK)
                    ps_s = sqp.tile([P, 2, CHUNK], f32, tag="sq", name="sq")
                    for s in range(2):
                        jb = 2 * m + s
                        jsl = slice(jb * P, (jb + 1) * P)
                        nc.tensor.matmul(ps_s[:, s, :], k_f8[:, :, jsl],
                                         qT2[:, :, isl], start=True,
                                         stop=True, perf_mode=DR)
                    e_pair = epool.tile([P, 2, CHUNK], f8, tag="e", name="e")
                    # e^{s/16 - 4}: the -4 keeps E in fp8's finite range
                    # and cancels exactly in the softmax normalization
                    nc.scalar.activation(e_pair[:], ps_s[:], AF.Exp,
                                         scale=float(C) ** -0.5,
                                         bias=neg4_sb[:])
                    if m == 2 and pending5[0] is not None:
                        pending5[0]()
                        pending5[0] = None
                    if prev is not None:
                        accums(*prev)
                    prev = (gp, e_pair)
                accums(*prev)
                pending5[0]()

    nc.compile()
    return nc


def _get_exec():
    if "fn" in _cache:
        return _cache["fn"], _cache["zfn"], _cache["in_names"]

    import jax
    import jax.numpy as jnp
    import ml_dtypes
    from jax.experimental.shard_map import shard_map
    from jax.sharding import Mesh, NamedSharding, PartitionSpec as PS

    from concourse import bass2jax, mybir

    try:
        jax.config.update("jax_compilation_cache_dir", "/tmp/jax_cc_cache")
        jax.config.update("jax_persistent_cache_min_compile_time_secs", 0.0)
    except Exception:
        pass

    nc = _build_nc()
    _cache["nc"] = nc
    bass2jax.install_neuronx_cc_hook()

    partition_name = (nc.partition_id_tensor.name
                      if nc.partition_id_tensor else None)
    in_names, out_names, out_avals = [], [], []
    for alloc in nc.m.functions[0].allocations:
        if not isinstance(alloc, mybir.MemoryLocationSet):
            continue
        name = alloc.memorylocations[0].name
        if alloc.kind == "ExternalInput":
            if name != partition_name:
                in_names.append(name)
        elif alloc.kind == "ExternalOutput":
            out_avals.append(jax.core.ShapedArray(
                tuple(alloc.tensor_shape), mybir.dt.np(alloc.dtype)))
            out_names.append(name)
    n_params = len(in_names)
    all_in_names = in_names + out_names
    if partition_name:
        all_in_names = all_in_names + [partition_name]

    def _body(*args):
        operands = list(args)
        if partition_name:
            operands.append(bass2jax.partition_id_tensor())
        outs = bass2jax._bass_exec_p.bind(
            *operands, out_avals=tuple(out_avals),
            in_names=tuple(all_in_names), out_names=tuple(out_names),
            lowering_input_output_aliases=(), sim_require_finite=True,
            sim_require_nnan=True, nc=nc)
        return tuple(outs)

    devices = np.asarray(jax.devices()[:NCORES]).reshape(B, 2)
    mesh = Mesh(devices, ("pair", "half"))
    spec_by_name = {"x": PS("pair"), "qpack": PS("half"), "wbig": PS(),
                    "wsml": PS(), "windt": PS()}
    in_specs = (tuple(spec_by_name[n] for n in in_names)
                + (PS(("pair", "half")), PS(("pair", "half"))))
    out_specs = (PS(("pair", "half")), PS(("pair", "half")))

    fn = jax.jit(
        shard_map(_body, mesh=mesh, in_specs=in_specs,
                  out_specs=out_specs, check_rep=False),
        donate_argnums=(n_params, n_params + 1), keep_unused=True)

    zsharding = NamedSharding(mesh, PS(("pair", "half")))
    zfn = jax.jit(
        lambda: (jnp.zeros((NCORES * C, HALF), ml_dtypes.bfloat16),
                 jnp.zeros((NCORES, HALF), np.float32)),
        out_shardings=(zsharding, zsharding))

    _cache["fn"] = fn
    _cache["zfn"] = zfn
    _cache["in_names"] = in_names
    return fn, zfn, in_names


def _pack_inputs(x, gn_w, gn_b, wq, bq, wk, bk, wv, bv, wp, bp):
    import ml_dtypes
    bfd = ml_dtypes.bfloat16
    f = np.float32
    asrt = lambda a: np.asarray(a, f)
    x = np.asarray(x, f).reshape(B * C, N)
    x_bf = x.astype(bfd)

    wq, wk, wv, wp = asrt(wq), asrt(wk), asrt(wv), asrt(wp)
    bq, bk, bv, bp = asrt(bq), asrt(bk), asrt(bv), asrt(bp)
    gn_w, gn_b = asrt(gn_w), asrt(gn_b)

    qpack = np.empty((2 * (C + 1), P), bfd)
    wqT = wq.T.astype(bfd)
    for j in range(2):
        qpack[j * (C + 1):j * (C + 1) + C] = wqT[:, j * P:(j + 1) * P]
        qpack[j * (C + 1) + C] = bq[j * P:(j + 1) * P].astype(bfd)

    wbig = np.empty((C, WBCOLS), bfd)
    wbig[:, WK0:WK0 + C] = wk.T.astype(bfd)
    wbig[:, WV0:WV0 + C] = wv.T.astype(bfd)
    wbig[:, WP0:WP0 + C] = wp.T.astype(bfd)

    wsml = np.zeros((C, WSCOLS), f)
    wsml[:, BK0] = bk
    wsml[:, BV0] = bv
    wsml[:, BP0] = bp
    wsml[:, GW0] = gn_w
    wsml[:, GB0] = gn_b
    ind = np.zeros((C, GROUPS), f)
    ind[np.arange(C), np.arange(C) // GSIZE] = 1.0
    wsml[:, IND0:IND0 + GROUPS] = ind
    windt = np.ascontiguousarray(ind.T)
    return x, x_bf, qpack, wbig, wsml, windt, bp


def _col_perm():
    # device column d (per half): chunk sc = d//512, r = 4*sc + (d%512)//128,
    # a = d%128 -> true column i = 16a + r
    d = np.arange(HALF)
    r = 4 * (d // CHUNK) + (d % CHUNK) // P
    a = d % P
    return 16 * a + r


def kernel(x, gn_w, gn_b, wq, bq, wk, bk, wv, bv, wp, bp):
    fn, zfn, in_names = _get_exec()
    x_f32, x_bf, qpack, wbig, wsml, windt, bp_f = _pack_inputs(
        x, gn_w, gn_b, wq, bq, wk, bk, wv, bv, wp, bp)
    arrs = {"x": x_bf, "qpack": qpack, "wbig": wbig, "wsml": wsml,
            "windt": windt}
    p_out, l_out = fn(*(arrs[n] for n in in_names), *zfn())
    # p_out: (8*C, HALF) bf16 unnormalized with permuted columns,
    # l_out: (8, HALF) f32; blocks ordered core = 2b + j. The host
    # un-permutes the columns and applies p/l + bp + residual.
    perm = _cache.setdefault("perm", _col_perm())
    p = np.asarray(p_out).astype(np.float32).reshape(B, 2, C, HALF)
    l = np.asarray(l_out).astype(np.float32).reshape(B, 2, 1, HALF)
    p = p / l + bp_f[None, None, :, None]
    out = np.empty((B, C, N), np.float32)
    for j in range(2):
        out[:, :, j * HALF + perm] = p[:, j]
    out += x_f32.reshape(B, C, N)
    return out.reshape(B, C, H, W)


# revision 14
# speedup vs baseline: 5403.0684x; 1.1110x over previous
"""AttnBlock on 8 Trainium2 NeuronCores via Bass/Tile.

Reference computation (shapes hardcoded): x (4, 256, 64, 64) f32,
GroupNorm(32 groups) -> q/k/v 1x1 conv -> HWxHW attention (with the
reference's raw-view reshape (C,N)->(N,C) for q and v) -> proj -> x + p.

Sharding: 8 cores = 4 batch elements x 2 query-halves, mesh (pair=4,
half=2). Core (b, j) handles batch b and attention rows n in
[j*2048, (j+1)*2048). The raw view means q_att rows [j*2048,(j+1)*2048)
depend only on wq rows [j*128,(j+1)*128), so each core computes: full
k/v, its half of qT, its half of the attention, and p columns
[j*2048,(j+1)*2048). No collectives.

GroupNorm is FOLDED into the q/k/v conv weights: h = scale_c*x + bias_c
per channel, so W@h = (W*scale)@x + W_s@(bias_c/scale). The per-channel
scale rides the weight converts (same op count as a plain convert), the
bias terms become tiny matmuls, and h is never materialized - the convs
consume x directly.

Key layout identity (N=4096=HW, C=256): q_att[n,c] = q[n//16, 256*(n%16)+c],
so  qT[c, 16a+r] = (x[:, 256r:256r+256].T @ wq_half_s.T)[c, a].
qT is stored COLUMN-PERMUTED as qT2[c, cb, 128r+a] so each PSUM->SBUF
copy lands contiguously; one scalar_tensor_tensor per (quad, cb) casts
8 matmul outputs at once while adding the folded q bias via a
partition-broadcast row. The attention then simply runs on permuted i
columns (chunk sc covers r in [4sc,4sc+4)) and the host un-permutes the
output columns during the unshard.

Attention is computed transposed: ST[j,i] = sum_c k[c,j]*qT[c,i], then
E = exp(ST/16 - 4) (scores are ~N(0,1): no max subtraction needed, and
the -4 centers E in fp8 range and cancels in the normalization),
h_attT[c,i] = sum_j v_att[j,c]*E[j,i] accumulated in PSUM over j-block
pairs. k/qT/E/v_att are fp8(e4m3) with K=256 packed [128,2,.] for
DoubleRow matmuls. The softmax denominator accumulates via M=1
ones-matmuls and ships to the host as a second tiny output; the host
performs p/l + bp during the unshard, so the device ships the
unnormalized wp @ h_attT in bf16 (bf16 is floating, so the larger
magnitude costs no relative precision). In the attention phase ACT does
nothing but the 64 exps (the exp chain is the co-bottleneck with the
PE's matmul stream), all PSUM->SBUF traffic runs on DVE, and the pair
pipeline is FLAT across chunk boundaries - the accumulation matmuls run
one pair behind the score matmuls everywhere, so the PE never idles at
a chunk edge waiting for exp.

Host I/O is minimized: x ships bf16, the big weights ship bf16
(wk|wv|wp transposed), one-hots f32, per-half q weights per half.
Outputs are the bf16 p-halves plus the f32 denominator rows.
"""

import numpy as np

B, C, H, W = 4, 256, 64, 64
N = H * W            # 4096 pixels
HALF = N // 2        # 2048 attention rows per core
GROUPS = 32
GSIZE = C // GROUPS  # 8 channels per group
EPS = 1e-5
NCORES = 8
P = 128              # partitions
NB = N // P          # 32 j-blocks of 128
SC = 4               # i super-chunks per core
CHUNK = 512          # i columns per chunk (one PSUM bank)
NPAIR = NB // 2      # 16 j-block pairs per chunk

# wbig column layout (bf16, C rows): wk.T | wv.T | wp.T
WK0, WV0, WP0 = 0, C, 2 * C
WBCOLS = 3 * C
# wsml column layout (f32, C rows)
BK0, BV0, BP0, GW0, GB0 = 0, 1, 2, 3, 4
IND0 = 5
WSCOLS = IND0 + GROUPS

_cache = {}


def _build_nc():
    import concourse.tile as tile
    from concourse import bacc, mybir

    f32 = mybir.dt.float32
    bf16 = mybir.dt.bfloat16
    f8 = mybir.dt.float8e4
    AF = mybir.ActivationFunctionType
    OP = mybir.AluOpType

    nc = bacc.Bacc("TRN2", target_bir_lowering=False, debug=False,
                   num_devices=NCORES)

    x_ap = nc.dram_tensor("x", [C, N], bf16, kind="ExternalInput").ap()
    qpack_ap = nc.dram_tensor("qpack", [C + 1, P], bf16,
                              kind="ExternalInput").ap()
    wbig_ap = nc.dram_tensor("wbig", [C, WBCOLS], bf16,
                             kind="ExternalInput").ap()
    wsml_ap = nc.dram_tensor("wsml", [C, WSCOLS], f32,
                             kind="ExternalInput").ap()
    windt_ap = nc.dram_tensor("windt", [GROUPS, C], f32,
                              kind="ExternalInput").ap()
    out_ap = nc.dram_tensor("out", [C, HALF], bf16, kind="ExternalOutput").ap()
    lout_ap = nc.dram_tensor("lout", [1, HALF], f32,
                             kind="ExternalOutput").ap()

    with tile.TileContext(nc) as tc:
        with (
            tc.tile_pool(name="persist", bufs=1) as persist,
            tc.tile_pool(name="small", bufs=4) as small,
            tc.tile_pool(name="epool", bufs=4) as epool,
            tc.tile_pool(name="htpool", bufs=4) as htpool,
            tc.tile_pool(name="opool", bufs=4) as opool,
        ):
            # constants first so nothing queues behind the big DMAs
            warm_w = persist.tile([P, P], bf16, tag="warmw", name="warmw")
            nc.vector.memset(warm_w[:], 1.0)
            ones_pair_f8 = persist.tile([P, 2, 16], f8, tag="ones_pair",
                                        name="ones_pair")
            nc.vector.memset(ones_pair_f8[:], 1.0)
            eps_sb = persist.tile([GROUPS, 1], f32, tag="eps", name="eps")
            nc.vector.memset(eps_sb[:], EPS)
            neg4_sb = persist.tile([P, 1], f32, tag="neg4", name="neg4")
            nc.vector.memset(neg4_sb[:], -4.0)

            # ---------- x + weights across the three DMA queues ----------
            # Bigger chunks (4KB per-partition lines) use the per-queue DMA
            # bandwidth better; the per-queue byte budget is balanced so
            # all of x lands at roughly the same time on every queue.
            x_sb = [persist.tile([P, N], bf16, tag=f"x{cb}", name=f"x{cb}")
                    for cb in range(2)]

            def xdma(eng, cb, c0, c1):
                eng.dma_start(x_sb[cb][:, c0:c1],
                              x_ap[cb * P:(cb + 1) * P, c0:c1])

            xdma(nc.sync, 0, 0, 2048)
            xdma(nc.scalar, 1, 0, 2048)
            xdma(nc.gpsimd, 0, 2048, 3072)
            xdma(nc.gpsimd, 1, 2048, 3072)
            xdma(nc.sync, 1, 3072, 4096)
            xdma(nc.scalar, 0, 3072, 4096)

            def rows(cb):
                return slice(cb * P, (cb + 1) * P)

            wbig_sb, qp_sb = [], []
            for cb in range(2):
                t = persist.tile([P, WBCOLS], bf16, tag=f"wb{cb}",
                                 name=f"wb{cb}")
                nc.gpsimd.dma_start(t[:], wbig_ap[rows(cb), :])
                wbig_sb.append(t)
                t = persist.tile([P, P], bf16, tag=f"qp{cb}", name=f"qp{cb}")
                nc.sync.dma_start(t[:], qpack_ap[rows(cb), :])
                qp_sb.append(t)
            bq_row_bf = persist.tile([1, P], bf16, tag="bqrow", name="bqrow")
            nc.sync.dma_start(bq_row_bf[:], qpack_ap[C:C + 1, :])
            wsml_sb = []
            for cb in range(2):
                t = persist.tile([P, WSCOLS], f32, tag=f"ws{cb}",
                                 name=f"ws{cb}")
                nc.scalar.dma_start(t[:], wsml_ap[rows(cb), :])
                wsml_sb.append(t)
            windt_sb = persist.tile([GROUPS, C], f32, tag="windt",
                                    name="windt")
            nc.scalar.dma_start(windt_sb[:], windt_ap[:, :])

            def wsml(cb, c0, c1=None):
                c1 = c0 + 1 if c1 is None else c1
                return wsml_sb[cb][:, c0:c1]

            wpt_bf = [wbig_sb[cb][:, WP0:WP0 + C] for cb in range(2)]

            DR = mybir.MatmulPerfMode.DoubleRow

            # persistent fp8 tensors
            k_f8 = persist.tile([P, 2, N], f8, tag="kf8", name="kf8")
            v_f8 = [persist.tile([P, N], f8, tag=f"vf8{cb}", name=f"vf8{cb}")
                    for cb in range(2)]
            v_att = persist.tile([P, NB // 2, 2, C], f8, tag="vatt",
                                 name="vatt")
            # permuted q_att.T: qT2[c', cb, 128r+a] = q_att.T[128cb+c', 16a+r]
            qT2 = persist.tile([P, 2, HALF], f8, tag="qT", name="qT")

            # ---------- pre-attention PSUM: four [P,2,512] tiles ---------
            # (smaller tiles because dependency tracking is per-tile: a
            # PSUM->SBUF copy only starts once ALL matmuls into its tile
            # are done, so deep small tiles keep the copies off the ring's
            # critical path)
            with tc.tile_pool(name="ps4", bufs=4, space="PSUM") as ps4:
                def quad():
                    return ps4.tile([P, 2, CHUNK], f32, tag="c2", name="c2")

                # dummy matmuls consuming each x chunk as it arrives keep
                # the HAM clock-gate at full rate into the convs
                def junk(cb, ch):
                    jq = quad()
                    nc.tensor.matmul(
                        jq[:, 0, :], warm_w[:],
                        x_sb[cb][:, ch * CHUNK:(ch + 1) * CHUNK],
                        start=True, stop=True)

                for rep in range(2):
                    for q in range(4):
                        for cb in range(2):
                            junk(cb, 2 * q + rep)

                # ---------- GroupNorm stats ----------
                m1m2 = []
                for cb in range(2):
                    xv = x_sb[cb].rearrange("p (s q) -> p s q", q=512)
                    stats = small.tile([P, 8, 6], f32, tag="bnstats",
                                       name="bnstats")
                    for s in range(8):
                        nc.vector.bn_stats(stats[:, s, :], xv[:, s, :])
                    mv = small.tile([P, 2], f32, tag="bnmv", name="bnmv")
                    nc.vector.bn_aggr(mv[:], stats[:])
                    mm12 = small.tile([P, 2], f32, tag="m1m2", name="m1m2")
                    nc.vector.tensor_copy(mm12[:, 0:1], mv[:, 0:1])
                    # E[x^2] = mean^2 + var in one fused op
                    nc.vector.scalar_tensor_tensor(
                        mm12[:, 1:2], mv[:, 0:1], mv[:, 0:1], mv[:, 1:2],
                        op0=OP.mult, op1=OP.add)
                    m1m2.append(mm12)

                gq = quad()
                # group sums: [32, 2] = sum over channels in group
                nc.tensor.matmul(gq[0:GROUPS, 0, 0:2],
                                 wsml(0, IND0, IND0 + GROUPS),
                                 m1m2[0][:], start=True, stop=False)
                nc.tensor.matmul(gq[0:GROUPS, 0, 0:2],
                                 wsml(1, IND0, IND0 + GROUPS),
                                 m1m2[1][:], start=False, stop=True)

                gstats = small.tile([GROUPS, 2], f32, tag="gstats",
                                    name="gstats")
                nc.vector.tensor_scalar_mul(gstats[:, 0:1],
                                            gq[0:GROUPS, 0, 0:1],
                                            1.0 / GSIZE)
                musq = small.tile([GROUPS, 1], f32, tag="gmusq", name="gmusq")
                nc.vector.tensor_mul(musq[:], gstats[:, 0:1], gstats[:, 0:1])
                # var = E[x^2]/8 - mean^2 in one fused op
                gvar = small.tile([GROUPS, 1], f32, tag="gvar", name="gvar")
                nc.vector.scalar_tensor_tensor(
                    gvar[:], gq[0:GROUPS, 0, 1:2], 1.0 / GSIZE, musq[:],
                    op0=OP.mult, op1=OP.subtract)
                gsd = small.tile([GROUPS, 1], f32, tag="gsd", name="gsd")
                nc.scalar.activation(gsd[:], gvar[:], AF.Sqrt, bias=eps_sb[:])
                nc.vector.reciprocal(gstats[:, 1:2], gsd[:])

                junk(0, 1)

                # scatter group stats to channels; per-channel fold params
                scale_c, bias2_bf = [], []
                sq2 = quad()
                for cb in range(2):
                    nc.tensor.matmul(sq2[:, cb, 0:2],
                                     windt_sb[:, cb * P:(cb + 1) * P],
                                     gstats[:], start=True, stop=True)
                    sc_ = small.tile([P, 1], f32, tag="scalec", name="scalec")
                    nc.vector.tensor_mul(sc_[:], sq2[:, cb, 1:2],
                                         wsml(cb, GW0))
                    rsc = small.tile([P, 1], f32, tag="rsc", name="rsc")
                    nc.vector.reciprocal(rsc[:], sc_[:])
                    # bias2 = bias_c/scale = gnb/scale - mean, fused; the
                    # folded weights then give W_s @ bias2 = W @ bias_c
                    b2b = small.tile([P, 1], bf16, tag="b2b", name="b2b")
                    nc.vector.scalar_tensor_tensor(
                        b2b[:], wsml(cb, GB0), rsc[:], sq2[:, cb, 0:1],
                        op0=OP.mult, op1=OP.subtract)
                    scale_c.append(sc_)
                    bias2_bf.append(b2b)

                junk(1, 1)

                # folded bf16 weights: W_s = W.T * scale_c (per partition)
                def fold(src_ap, cols, tag, cb, eng):
                    t = persist.tile([P, cols], bf16, tag=tag, name=tag)
                    if eng == "dve":
                        nc.vector.tensor_scalar_mul(t[:], src_ap,
                                                    scale_c[cb][:])
                    else:
                        nc.scalar.activation(t[:], src_ap, AF.Identity,
                                             scale=scale_c[cb][:])
                    return t

                wqt_s = [fold(qp_sb[cb][:], P, f"wqs{cb}", cb,
                              "dve" if cb == 0 else "act")
                         for cb in range(2)]
                wkt_s = [fold(wbig_sb[cb][:, WK0:WK0 + C], C, f"wks{cb}", cb,
                              "dve" if cb == 0 else "act")
                         for cb in range(2)]
                wvt_s = [fold(wbig_sb[cb][:, WV0:WV0 + C], C, f"wvs{cb}", cb,
                              "dve" if cb == 0 else "act")
                         for cb in range(2)]

                junk(0, 3)

                # bias folds: b' = b + W @ bias_c = b + W_s @ bias2
                bq2 = {0: quad(), 1: quad()}
                for s, wt in ((0, wkt_s), (1, wvt_s)):
                    for cob in range(2):
                        psl = bq2[s][:, cob, 0:1]
                        nc.tensor.matmul(psl, wt[0][:, cob * P:(cob + 1) * P],
                                         bias2_bf[0][:], start=True,
                                         stop=False)
                        nc.tensor.matmul(psl, wt[1][:, cob * P:(cob + 1) * P],
                                         bias2_bf[1][:], start=False,
                                         stop=True)
                bq3 = quad()
                nc.tensor.matmul(bq3[0:1, 0, 0:P], bias2_bf[0][:], wqt_s[0][:],
                                 start=True, stop=False)
                nc.tensor.matmul(bq3[0:1, 0, 0:P], bias2_bf[1][:], wqt_s[1][:],
                                 start=False, stop=True)

                bkp, bvp = [], []
                for cob in range(2):
                    t = small.tile([P, 1], f32, tag="bkp", name="bkp")
                    nc.vector.tensor_add(t[:], bq2[0][:, cob, 0:1],
                                         wsml(cob, BK0))
                    bkp.append(t)
                    t = small.tile([P, 1], f32, tag="bvp", name="bvp")
                    nc.vector.tensor_add(t[:], bq2[1][:, cob, 0:1],
                                         wsml(cob, BV0))
                    bvp.append(t)
                # bq' row replicated x8 then partition-broadcast, so one
                # STT per (quad, cb) adds the bias to 8 slots at once
                bq_row8 = small.tile([1, 8, P], f32, tag="bqp", name="bqp")
                for rep in range(8):
                    nc.vector.tensor_add(bq_row8[:, rep, :],
                                         bq3[0:1, 0, 0:P], bq_row_bf[:])
                bq_bc8 = persist.tile([P, 8 * P], f32, tag="bqbc",
                                      name="bqbc")
                nc.gpsimd.partition_broadcast(
                    bq_bc8[:], bq_row8.rearrange("o r p -> o (r p)"))

                junk(1, 3)

                # ---------- k, v convs (K=256 via 2 bf16 matmuls) --------
                # 2 chunks per PSUM tile, one merged 1024-col copy per
                # tile, alternating DVE/ACT so each engine drains at half
                # the matmul rate
                def conv_full(wt, b_sb, dst, e0):
                    for cob in range(2):
                        for t4 in range(4):
                            ps = quad()
                            for s in range(2):
                                ch = 2 * t4 + s
                                sl = slice(ch * CHUNK, (ch + 1) * CHUNK)
                                nc.tensor.matmul(
                                    ps[:, s, :],
                                    wt[0][:, cob * P:(cob + 1) * P],
                                    x_sb[0][:, sl], start=True, stop=False)
                                nc.tensor.matmul(
                                    ps[:, s, :],
                                    wt[1][:, cob * P:(cob + 1) * P],
                                    x_sb[1][:, sl], start=False, stop=True)
                            sl = slice(2 * t4 * CHUNK, (2 * t4 + 2) * CHUNK)
                            if t4 % 2 == e0:
                                nc.vector.tensor_scalar_add(
                                    dst(cob, sl), ps[:, 0:2, :], b_sb[cob][:])
                            else:
                                nc.scalar.activation(
                                    dst(cob, sl), ps[:, 0:2, :],
                                    AF.Identity, bias=b_sb[cob][:])

                conv_full(wkt_s, bkp, lambda cob, sl: k_f8[:, cob, sl], 0)
                conv_full(wvt_s, bvp, lambda cob, sl: v_f8[cob][:, sl], 1)

                # v_att[j, c] = v[j//16, 256*(j%16)+c]; [j', pair, jlo, c]
                # so a [128, 2, 128] DoubleRow stationary covers two
                # j-blocks. Spread over three DMA queues.
                for jb in range(NB):
                    cb = jb // 16
                    p0 = (jb % 16) * 8
                    src = v_f8[cb][p0:p0 + 8, :].rearrange(
                        "p (r c) -> p r c", c=C)
                    eng = (nc.sync, nc.scalar, nc.gpsimd)[jb % 3]
                    eng.dma_start(v_att[:, jb // 2, jb % 2, :], src)

                # ---------- qT2: permuted q_att.T for this core's half ---
                # qT2[m, cb, 128r+a] = qconv[a, 256r+128cb+m]. Eight
                # matmul pairs fill a PSUM tile, then one big STT casts
                # them to fp8 while adding the bias row in a single op.
                for qd in range(2):
                    for cb in range(2):
                        ps = quad()
                        for k8 in range(8):
                            r = 8 * qd + k8
                            sl = slice(256 * r + cb * P,
                                       256 * r + (cb + 1) * P)
                            psl = ps[:, k8 // 4,
                                     (k8 % 4) * P:(k8 % 4 + 1) * P]
                            nc.tensor.matmul(psl, x_sb[0][:, sl], wqt_s[0][:],
                                             start=True, stop=False)
                            nc.tensor.matmul(psl, x_sb[1][:, sl], wqt_s[1][:],
                                             start=False, stop=True)
                        nc.vector.scalar_tensor_tensor(
                            qT2[:, cb, qd * 1024:(qd + 1) * 1024],
                            ps.rearrange("p s q -> p (s q)"),
                            1.0, bq_bc8[:], op0=OP.mult, op1=OP.add)

            # ---------- attention + projection ----------
            # flat pair pipeline across all 4 chunks: scores(p) -> exp(p)
            # -> accums(p-1); chunk bookkeeping (accumulator drain, wp
            # projection, output DMA) is spliced into the stream so the
            # PE and ACT never wait at a chunk boundary.
            with (
                tc.tile_pool(name="sqp", bufs=2, space="PSUM") as sqp,
                tc.tile_pool(name="hacc", bufs=3, space="PSUM") as hacc,
                tc.tile_pool(name="lacc", bufs=1, space="PSUM") as lacc,
            ):
                def make_stage5a(isl, hps, lp):
                    """Drain the accumulators right after their last matmul
                    so the PSUM banks recycle before the next chunk's
                    accums; the denominator row bounces through SBUF."""
                    l_sb = small.tile([1, CHUNK], f32, tag="lsb", name="lsb")
                    nc.vector.tensor_copy(l_sb[:], lp[:])
                    nc.sync.dma_start(lout_ap[0:1, isl], l_sb[:])
                    hT = [htpool.tile([P, CHUNK], bf16, tag="hT", name="hT")
                          for _ in range(2)]
                    nc.vector.tensor_copy(hT[0][:], hps[0][:])
                    nc.vector.tensor_copy(hT[1][:], hps[1][:])
                    return hT

                def make_stage5b(isl, hT):
                    def stage5b():
                        # the projection borrows one slot of the scores
                        # ring (same tag/shape) instead of its own bank
                        pp = sqp.tile([P, 2, CHUNK], f32, tag="sq",
                                      name="sq")
                        for cob in range(2):
                            nc.tensor.matmul(
                                pp[:, cob, :],
                                wpt_bf[0][:, cob * P:(cob + 1) * P],
                                hT[0][:], start=True, stop=False)
                            nc.tensor.matmul(
                                pp[:, cob, :],
                                wpt_bf[1][:, cob * P:(cob + 1) * P],
                                hT[1][:], start=False, stop=True)
                            o_t = opool.tile([P, CHUNK], bf16, tag="ot",
                                             name="ot")
                            nc.vector.tensor_copy(o_t[:], pp[:, cob, :])
                            nc.sync.dma_start(
                                out_ap[cob * P:(cob + 1) * P, isl], o_t[:])
                    return stage5b

                state = {"hps": None, "lp": None}
                pending5 = [None]

                def accums(gp, e_pair):
                    """h_attT and softmax-denominator accumulation for one
                    j-block pair of chunk gp//NPAIR (DoubleRow, K=256)."""
                    pair = gp % NPAIR
                    st, sp = (pair == 0), (pair == NPAIR - 1)
                    if st:
                        state["hps"] = [
                            hacc.tile([P, CHUNK], f32, tag="hacc",
                                      name="hacc") for _ in range(2)]
                        state["lp"] = lacc.tile([1, CHUNK], f32, tag="lacc",
                                                name="lacc")
                    hps, lp = state["hps"], state["lp"]
                    nc.tensor.matmul(hps[0][:], v_att[:, pair, :, 0:P],
                                     e_pair[:], start=st, stop=sp,
                                     perf_mode=DR)
                    nc.tensor.matmul(hps[1][:], v_att[:, pair, :, P:C],
                                     e_pair[:], start=st, stop=sp,
                                     perf_mode=DR)
                    nc.tensor.matmul(lp[:], ones_pair_f8[:, :, 0:1],
                                     e_pair[:], start=st, stop=sp,
                                     perf_mode=DR)
                    if sp:
                        sc = gp // NPAIR
                        isl = slice(sc * CHUNK, (sc + 1) * CHUNK)
                        hT = make_stage5a(isl, hps, lp)
                        pending5[0] = make_stage5b(isl, hT)

                # accums run TWO pairs behind the scores: exp(p) spans more
                # than one pair of PE work, so a depth-1 pipeline would
                # stall the accums on the activation every pair.
                pipe = []
                for gp in range(SC * NPAIR):
                    sc, m = divmod(gp, NPAIR)
                    isl = slice(sc * CHUNK, (sc + 1) * CHUNK)
                    ps_s = sqp.tile([P, 2, CHUNK], f32, tag="sq", name="sq")
                    for s in range(2):
                        jb = 2 * m + s
                        jsl = slice(jb * P, (jb + 1) * P)
                        nc.tensor.matmul(ps_s[:, s, :], k_f8[:, :, jsl],
                                         qT2[:, :, isl], start=True,
                                         stop=True, perf_mode=DR)
                    e_pair = epool.tile([P, 2, CHUNK], f8, tag="e", name="e")
                    # e^{s/16 - 4}: the -4 keeps E in fp8's finite range
                    # and cancels exactly in the softmax normalization
                    nc.scalar.activation(e_pair[:], ps_s[:], AF.Exp,
                                         scale=float(C) ** -0.5,
                                         bias=neg4_sb[:])
                    if m == 3 and pending5[0] is not None:
                        pending5[0]()
                        pending5[0] = None
                    pipe.append((gp, e_pair))
                    if len(pipe) > 2:
                        accums(*pipe.pop(0))
                for it in pipe:
                    accums(*it)
                pending5[0]()

    nc.compile()
    return nc


def _get_exec():
    if "fn" in _cache:
        return _cache["fn"], _cache["zfn"], _cache["in_names"]

    import jax
    import jax.numpy as jnp
    import ml_dtypes
    from jax.experimental.shard_map import shard_map
    from jax.sharding import Mesh, NamedSharding, PartitionSpec as PS

    from concourse import bass2jax, mybir

    try:
        jax.config.update("jax_compilation_cache_dir", "/tmp/jax_cc_cache")
        jax.config.update("jax_persistent_cache_min_compile_time_secs", 0.0)
    except Exception:
        pass

    nc = _build_nc()
    _cache["nc"] = nc
    bass2jax.install_neuronx_cc_hook()

    partition_name = (nc.partition_id_tensor.name
                      if nc.partition_id_tensor else None)
    in_names, out_names, out_avals = [], [], []
    for alloc in nc.m.functions[0].allocations:
        if not isinstance(alloc, mybir.MemoryLocationSet):
            continue
        name = alloc.memorylocations[0].name
        if alloc.kind == "ExternalInput":
            if name != partition_name:
                in_names.append(name)
        elif alloc.kind == "ExternalOutput":
            out_avals.append(jax.core.ShapedArray(
                tuple(alloc.tensor_shape), mybir.dt.np(alloc.dtype)))
            out_names.append(name)
    n_params = len(in_names)
    all_in_names = in_names + out_names
    if partition_name:
        all_in_names = all_in_names + [partition_name]

    def _body(*args):
        operands = list(args)
        if partition_name:
            operands.append(bass2jax.partition_id_tensor())
        outs = bass2jax._bass_exec_p.bind(
            *operands, out_avals=tuple(out_avals),
            in_names=tuple(all_in_names), out_names=tuple(out_names),
            lowering_input_output_aliases=(), sim_require_finite=True,
            sim_require_nnan=True, nc=nc)
        return tuple(outs)

    devices = np.asarray(jax.devices()[:NCORES]).reshape(B, 2)
    mesh = Mesh(devices, ("pair", "half"))
    spec_by_name = {"x": PS("pair"), "qpack": PS("half"), "wbig": PS(),
                    "wsml": PS(), "windt": PS()}
    in_specs = (tuple(spec_by_name[n] for n in in_names)
                + (PS(("pair", "half")), PS(("pair", "half"))))
    out_specs = (PS(("pair", "half")), PS(("pair", "half")))

    fn = jax.jit(
        shard_map(_body, mesh=mesh, in_specs=in_specs,
                  out_specs=out_specs, check_rep=False),
        donate_argnums=(n_params, n_params + 1), keep_unused=True)

    zsharding = NamedSharding(mesh, PS(("pair", "half")))
    zfn = jax.jit(
        lambda: (jnp.zeros((NCORES * C, HALF), ml_dtypes.bfloat16),
                 jnp.zeros((NCORES, HALF), np.float32)),
        out_shardings=(zsharding, zsharding))

    _cache["fn"] = fn
    _cache["zfn"] = zfn
    _cache["in_names"] = in_names
    return fn, zfn, in_names


def _pack_inputs(x, gn_w, gn_b, wq, bq, wk, bk, wv, bv, wp, bp):
    import ml_dtypes
    bfd = ml_dtypes.bfloat16
    f = np.float32
    asrt = lambda a: np.asarray(a, f)
    x = np.asarray(x, f).reshape(B * C, N)
    x_bf = x.astype(bfd)

    wq, wk, wv, wp = asrt(wq), asrt(wk), asrt(wv), asrt(wp)
    bq, bk, bv, bp = asrt(bq), asrt(bk), asrt(bv), asrt(bp)
    gn_w, gn_b = asrt(gn_w), asrt(gn_b)

    qpack = np.empty((2 * (C + 1), P), bfd)
    wqT = wq.T.astype(bfd)
    for j in range(2):
        qpack[j * (C + 1):j * (C + 1) + C] = wqT[:, j * P:(j + 1) * P]
        qpack[j * (C + 1) + C] = bq[j * P:(j + 1) * P].astype(bfd)

    wbig = np.empty((C, WBCOLS), bfd)
    wbig[:, WK0:WK0 + C] = wk.T.astype(bfd)
    wbig[:, WV0:WV0 + C] = wv.T.astype(bfd)
    wbig[:, WP0:WP0 + C] = wp.T.astype(bfd)

    wsml = np.zeros((C, WSCOLS), f)
    wsml[:, BK0] = bk
    wsml[:, BV0] = bv
    wsml[:, BP0] = bp
    wsml[:, GW0] = gn_w
    wsml[:, GB0] = gn_b
    ind = np.zeros((C, GROUPS), f)
    ind[np.arange(C), np.arange(C) // GSIZE] = 1.0
    wsml[:, IND0:IND0 + GROUPS] = ind
    windt = np.ascontiguousarray(ind.T)
    return x, x_bf, qpack, wbig, wsml, windt, bp


def _col_perm():
    # device column d (per half): chunk sc = d//512, r = 4*sc + (d%512)//128,
    # a = d%128 -> true column i = 16a + r
    d = np.arange(HALF)
    r = 4 * (d // CHUNK) + (d % CHUNK) // P
    a = d % P
    return 16 * a + r


def kernel(x, gn_w, gn_b, wq, bq, wk, bk, wv, bv, wp, bp):
    fn, zfn, in_names = _get_exec()
    x_f32, x_bf, qpack, wbig, wsml, windt, bp_f = _pack_inputs(
        x, gn_w, gn_b, wq, bq, wk, bk, wv, bv, wp, bp)
    arrs = {"x": x_bf, "qpack": qpack, "wbig": wbig, "wsml": wsml,
            "windt": windt}
    p_out, l_out = fn(*(arrs[n] for n in in_names), *zfn())
    # p_out: (8*C, HALF) bf16 unnormalized with permuted columns,
    # l_out: (8, HALF) f32; blocks ordered core = 2b + j. The host
    # un-permutes the columns and applies p/l + bp + residual.
    perm = _cache.setdefault("perm", _col_perm())
    p = np.asarray(p_out).astype(np.float32).reshape(B, 2, C, HALF)
    l = np.asarray(l_out).astype(np.float32).reshape(B, 2, 1, HALF)
    p = p / l + bp_f[None, None, :, None]
    out = np.empty((B, C, N), np.float32)
    for j in range(2):
        out[:, :, j * HALF + perm] = p[:, j]
    out += x_f32.reshape(B, C, N)
    return out.reshape(B, C, H, W)
